# revision 10
# baseline (speedup 1.0000x reference)
"""ANFIS (M=512, F=2, R=M^2, B=256) distributed Bass kernel for 8 TRN2 NeuronCores.

Math restructuring: with mem0[b,i] = gauss(x[b,0]; mean0[i], sig0[i]) and
mem1[b,j] = gauss(x[b,1]; mean1[j], sig1[j]), the reference output is

  out[b] = (x0[b]*q0[b] + x1[b]*q1[b] + q2[b]) / (s0[b]*s1[b])

where q_W[b] = mem0[b,:] @ W @ mem1[b,:]^T for W in {cw0, cw1, cb} reshaped
to [M, M], s0 = sum_i mem0, s1 = sum_j mem1.  This avoids materializing any
[B, R] tensor.

Sharding: 8 cores = 4 i-chunks (128 rows) x 2 j-halves (256 cols) of the
[M, M] weight matrices.  Each core emits partial numerator / denominator
factors [256, 2]; the host sums partials (all linear) and divides.

Per-core pipeline:
  mem0^T [i=128, b=256] = exp(-((x0b - mean0[i]) * (1/sig0[i]))^2)
     x0b: x0 broadcast along partitions (DMA broadcast), column scalars.
  mem1 [b=128, j=256] per b-half: rank-2 outer-product matmul in bf16
     (t = x1[b]*isig1[j] - mean1[j]*isig1[j]), isig1 row via exp(-ln(sig)).
  U [b, 769] = mem0_chunk @ [W0 | W1 | Wb | 1]  (bf16 matmul, f32 psum)
  q_Wx[b] = sum_j (U_W * xcol) * mem1  (scalar_tensor_tensor accum)
  out rows: num = q0x + q1x + q2, den = U[:,768] * s1.
"""

import os
import numpy as np

import concourse.bacc as bacc
import concourse.mybir as mybir
import concourse.tile as tile
from concourse.bass_utils import run_bass_kernel_spmd

try:
    import ml_dtypes
    BF16_NP = ml_dtypes.bfloat16
except ImportError:  # pragma: no cover
    BF16_NP = None

M = 512
B = 256
N_CORES = 8
IC = 4  # i-chunks
JHALF = 2  # j-halves
MI = M // IC  # 128 rows of W per core
MJ = M // JHALF  # 256 cols of W per core
NW = 3 * MJ + 1  # 769 = W0|W1|Wb|ones

F32 = mybir.dt.float32
BF16 = mybir.dt.bfloat16

_cache = {}


def _build():
    """Per-core SPMD graph (identical on all 8 cores; data differs)."""
    nc = bacc.Bacc("TRN2", target_bir_lowering=False, debug=False, num_devices=N_CORES)

    # cols [128, 6]: mean0c | sigma0c | x0h0 | x0h1 | x1h0 | x1h1
    # rows [1, 768]: x0(256) | mean1h(256) | sigma1h(256)
    # lhsb bf16 [1, 384]: x1(256) | -ones(128)
    # w bf16 [128, 769]: W0 | W1 | Wb | ones
    cols_ext = nc.declare_dram_parameter("cols", [128, 6], F32, isOutput=False)
    rows_ext = nc.declare_dram_parameter("rows", [1, 768], F32, isOutput=False)
    lhsb_ext = nc.declare_dram_parameter("lhsb", [1, B + 128], BF16, isOutput=False)
    w_ext = nc.declare_dram_parameter("w", [MI, NW], BF16, isOutput=False)
    out_ext = nc.declare_dram_parameter("out", [B, 2], F32, isOutput=True)

    mult = mybir.AluOpType.mult
    add = mybir.AluOpType.add
    sub = mybir.AluOpType.subtract
    EXP = mybir.ActivationFunctionType.Exp
    LN = mybir.ActivationFunctionType.Ln
    SQ = mybir.ActivationFunctionType.Square

    with tile.TileContext(nc) as tc:
        with (
            tc.tile_pool(name="const", bufs=1) as cp,
            tc.tile_pool(name="work", bufs=2) as wp,
            tc.tile_pool(name="psum", bufs=2, space="PSUM") as pp,
        ):
            cols = cp.tile([128, 6], F32)
            nc.sync.dma_start(cols[:], cols_ext[:])
            rows = cp.tile([1, 768], F32)
            nc.sync.dma_start(rows[:], rows_ext[:])
            lhsb = cp.tile([1, B + 128], BF16)
            nc.sync.dma_start(lhsb[:], lhsb_ext[:])
            w = cp.tile([MI, NW], BF16)
            nc.sync.dma_start(w[:], w_ext[:])
            x0b = cp.tile([128, B], F32)
            nc.sync.dma_start(x0b[:], rows_ext[0:1, 0:B].broadcast_to([128, B]))

            # isig0 column
            isig0 = cp.tile([128, 1], F32)
            nc.vector.reciprocal(isig0[:], cols[:, 1:2])

            # vb rows (bf16): [isig1(256) ; mean1*isig1(256)]
            lns = wp.tile([1, MJ], F32, tag="lns")
            nc.scalar.activation(lns[:], rows[0:1, 512:768], LN)
            vb = cp.tile([1, 2 * MJ], BF16)
            nc.scalar.activation(vb[0:1, 0:MJ], lns[:], EXP, scale=-1.0)
            nc.vector.tensor_tensor(vb[0:1, MJ:2 * MJ], rows[0:1, 256:512], vb[0:1, 0:MJ], mult)

            # mem0^T [i=128, b=256] in bf16
            tsa = wp.tile([128, B], F32, tag="tsa")
            nc.vector.tensor_scalar(tsa[:], x0b[:], cols[:, 0:1], isig0[:], sub, mult)
            sqa = wp.tile([128, B], F32, tag="sqa")
            nc.scalar.activation(sqa[:], tsa[:], SQ)
            m0t = cp.tile([128, B], BF16)
            nc.scalar.activation(m0t[:], sqa[:], EXP, scale=-1.0)

            for h in range(2):
                bh = slice(h * 128, (h + 1) * 128)
                # mem1 [b=128, j=256]: rank-2 bf16 matmul
                tb = pp.tile([128, MJ], F32, tag="tb")
                nc.tensor.matmul(tb[:], lhsb[0:1, bh], vb[0:1, 0:MJ], start=True, stop=False)
                nc.tensor.matmul(tb[:], lhsb[0:1, B:B + 128], vb[0:1, MJ:2 * MJ], start=False, stop=True)
                sqb = wp.tile([128, MJ], F32, tag="sqb")
                nc.scalar.activation(sqb[:], tb[:], SQ)
                m1 = wp.tile([128, MJ], F32, tag="m1")
                s1 = wp.tile([128, 1], F32, tag="s1")
                nc.scalar.activation(m1[:], sqb[:], EXP, scale=-1.0, accum_out=s1[:])

                # U[b, :] = mem0[b, chunk] @ [W0 | W1 | Wb | 1]
                u = pp.tile([128, NW], F32, tag="u")
                nc.tensor.matmul(u[:, 0:512], m0t[:, bh], w[:, 0:512], start=True, stop=True)
                nc.tensor.matmul(u[:, 512:NW], m0t[:, bh], w[:, 512:NW], start=True, stop=True)

                # q_Wx[b] = sum_j (U_W[b,j] * xw[b]) * mem1[b,j], fused scale
                qs = []
                for wi, xcol in ((0, cols[:, 2 + h:3 + h]), (1, cols[:, 4 + h:5 + h]), (2, None)):
                    scr = wp.tile([128, MJ], F32, tag="scr")
                    q = wp.tile([128, 1], F32, tag=f"q{wi}")
                    nc.vector.scalar_tensor_tensor(
                        scr[:], u[:, wi * MJ:(wi + 1) * MJ],
                        xcol if xcol is not None else 1.0,
                        m1[:], mult, mult, accum_out=q[:],
                    )
                    qs.append(q)

                res = wp.tile([128, 2], F32, tag="res")
                # den = s0 * s1   (s0 = ones-column of U)
                nc.vector.tensor_tensor(res[:, 1:2], u[:, 768:769], s1[:], mult)
                # num = q0x + q1x + q2
                nc.vector.tensor_scalar(res[:, 0:1], qs[0][:], qs[1][:], qs[2][:], add, add)

                nc.sync.dma_start(out_ext[bh, :], res[:])

    nc.compile()
    return nc


def _shard_inputs(x, mean, sigma, cw, cb):
    x = np.ascontiguousarray(x, np.float32)
    cwr = np.ascontiguousarray(cw, np.float32).reshape(M, M, 2)
    cbr = np.ascontiguousarray(cb, np.float32).reshape(M, M)
    lhsb = np.concatenate([x[:, 1], -np.ones(128, np.float32)])[None, :].astype(BF16_NP)
    ones_col = np.ones((MI, 1), np.float32)
    in_maps = []
    for c in range(N_CORES):
        ic, jh = c % IC, c // IC
        rows_s = slice(ic * MI, (ic + 1) * MI)
        cols_s = slice(jh * MJ, (jh + 1) * MJ)
        w = np.concatenate(
            [cwr[rows_s, cols_s, 0], cwr[rows_s, cols_s, 1], cbr[rows_s, cols_s], ones_col],
            axis=1, dtype=np.float32,
        ).astype(BF16_NP)
        colsv = np.stack([
            mean[0, rows_s], sigma[0, rows_s],
            x[0:128, 0], x[128:256, 0], x[0:128, 1], x[128:256, 1],
        ], axis=1)
        rowsv = np.concatenate([x[:, 0], mean[1, cols_s], sigma[1, cols_s]])[None, :]
        in_maps.append({
            "cols": np.ascontiguousarray(colsv, dtype=np.float32),
            "rows": np.ascontiguousarray(rowsv, dtype=np.float32),
            "lhsb": np.ascontiguousarray(lhsb),
            "w": np.ascontiguousarray(w),
        })
    return in_maps


def _ensure_ntff_hook():
    """The agent image's antenv lacks axon_hooks; build it from the boot
    helpers so run_bass_kernel_spmd(trace=True) can capture NTFF profiles."""
    import sys
    import types

    try:
        from antenv.axon_hooks import get_axon_ntff_profile_hook  # noqa: F401
        return
    except ImportError:
        pass
    mod = types.ModuleType("antenv.axon_hooks")
    holder = {}
    mod.set_axon_ntff_profile_hook = lambda h: holder.__setitem__("h", h)
    mod.get_axon_ntff_profile_hook = lambda: holder.get("h")
    try:
        from trn_agent_boot.trn_boot import _ntff_profile_via_ctypes

        hook = _ntff_profile_via_ctypes("/opt/axon/libaxon_pjrt.so")
        if hook is not None:
            holder["h"] = hook
    except Exception:
        pass
    sys.modules["antenv.axon_hooks"] = mod
    import antenv

    antenv.axon_hooks = mod


def run(inputs, trace=False, trace_kwargs=None):
    if trace:
        _ensure_ntff_hook()
    if "nc" not in _cache:
        _cache["nc"] = _build()
    nc = _cache["nc"]
    in_maps = _shard_inputs(**inputs)
    res = run_bass_kernel_spmd(
        nc, in_maps, core_ids=list(range(N_CORES)),
        trace=trace, **(trace_kwargs or {}),
    )
    outs = np.stack([r["out"] for r in res.results])  # [8, 256, 2]
    num = outs[:, :, 0].sum(axis=0)
    den = outs[:, :, 1].sum(axis=0)
    out = (num / den).astype(np.float32)[:, None]
    return out, res


def kernel(x, mean, sigma, cw, cb):
    out, _ = run(
        {"x": x, "mean": mean, "sigma": sigma, "cw": cw, "cb": cb},
        trace=bool(os.environ.get("ANFIS_TRACE")),
    )
    return out
